# revision 8
# baseline (speedup 1.0000x reference)
"""Trainium2 Bass kernel for the fp8 (e4m3fn fake-quant) SwiGLU MLP.

  x_q = e4m3fn(x / inp_scale)
  w1o = (x_q @ w1_fp8.T) * inp_scale * w1_scale
  w3o = (x_q @ w3_fp8.T) * inp_scale * w3_scale
  fused = silu(w1o) * w3o
  f_q = e4m3fn(fused / inp_scale)            # saturates to +-448, NaN above 464
  out = (f_q @ w2_fp8.T) * inp_scale * w2_scale

Strategy: token-shard across the 8 cores (512 tokens each) so there are no
collectives; every core streams the full (fp8, 1-byte) weights from HBM,
which hides easily under the compute at this size.  All three matmuls run
on the PE array in fp8 with perf_mode=DoubleRow (2 fp8 MACs/cell/cycle).

TRN2's fp8e4 format differs from OCP e4m3fn: max normal is 240 (not 448),
with Inf/NaN above.  Weights (which reach +-448) are therefore pre-halved on
the host -- every e4m3fn grid point /2 is exactly representable in TRN fp8e4
-- and the 2x is folded into the dequant scales.  The on-chip e4m3fn
quantization of `fused` uses the same trick: cast fused/2 to TRN fp8
(identical grid + identical RNE up to the 448/2=224 saturation point), then
overwrite entries with |fused/2| > 232 with NaN to replicate the ml_dtypes
overflow-to-NaN behaviour the reference exhibits on CPU.
"""

from contextlib import ExitStack

import ml_dtypes
import numpy as np

E4FN = ml_dtypes.float8_e4m3fn  # OCP: max 448, overflow->NaN
E4 = ml_dtypes.float8_e4m3      # TRN fp8e4: max 240, has Inf/NaN

P = 128

# Full problem shape (hardcoded per the harness contract).
T_FULL, HID_DIM, FFN_DIM = 4096, 4096, 14336
N_CORES = 8


def build_program(TSH, H, F, HID, c_sig, c_prod, thresh, c_out, nan_const=float("nan"),
                  debug_fq=False):
    """Build the single-core Bass/Tile program.

    DRAM I/O (names are the in_map keys):
      xq  [P, H//256, 2, TSH]      fp8e4  x^T in DoubleRow chunk layout
      w1  [F//P, P, H//256, 2, P]  fp8e4  (w1/2)^T strips per 128-wide f tile
      w3  like w1
      w2  [HID//P, P, F//256, 2, P] fp8e4 (w2/2)^T strips per 128-wide hid tile
      out [HID, TSH] f32           output^T for this core's tokens

    Per f-tile (128 ffn rows x TSH tokens), with psum1/psum2 the raw PE
    accumulations of the halved weights against x_q:
      sig  = Sigmoid(c_sig * psum1)          c_sig  = 2*xdiv*inp*s1
      t    = (psum1 * c_prod) * psum2        c_prod = c_sig*c3/(2*inp)
      h    = t * sig                         ( = fused/(2*inp) )
      fq   = trn_fp8(h); fq[|h| > thresh] = NaN        thresh = 232
    Then out_tile = psum3 * c_out with c_out = 4*xdiv?*... (see kernel()).
    """
    import concourse.bass as bass  # noqa: F401
    import concourse.mybir as mybir
    import concourse.tile as tile
    from concourse import bacc
    from concourse.alu_op_type import AluOpType

    dt = mybir.dt
    HCH = H // 256    # contraction chunks for mm1/mm2
    FT = F // P       # ffn tiles
    FCH = F // 256    # contraction chunks for mm3
    HT = HID // P     # output hid tiles
    assert H % 256 == 0 and F % 256 == 0 and HID % P == 0

    DR = mybir.MatmulPerfMode.DoubleRow
    AFT = mybir.ActivationFunctionType

    nc = bacc.Bacc(None, target_bir_lowering=False, debug=False)
    with tile.TileContext(nc) as tc, ExitStack() as ctx:
        dram = ctx.enter_context(tc.tile_pool(name="dram", bufs=1, space="DRAM"))
        xq_d = dram.tile([P, HCH, 2, TSH], dt.float8e4, kind="ExternalInput",
                         name="xq", uniquify=False)
        w1_d = dram.tile([FT, P, HCH, 2, P], dt.float8e4, kind="ExternalInput",
                         name="w1", uniquify=False)
        w3_d = dram.tile([FT, P, HCH, 2, P], dt.float8e4, kind="ExternalInput",
                         name="w3", uniquify=False)
        w2_d = dram.tile([HT, P, FCH, 2, P], dt.float8e4, kind="ExternalInput",
                         name="w2", uniquify=False)
        out_d = dram.tile([HID, TSH], dt.float32, kind="ExternalOutput",
                          name="out", uniquify=False)

        const = ctx.enter_context(tc.tile_pool(name="const", bufs=1))
        nan_t = const.tile([P, TSH], dt.float8e4)
        nc.vector.memset(nan_t[:], nan_const)

        xq_pool = ctx.enter_context(tc.tile_pool(name="xq_pool", bufs=1))
        xq_s = xq_pool.tile([P, HCH, 2, TSH], dt.float8e4)
        nc.sync.dma_start(xq_s[:], xq_d[:])

        fq_pool = ctx.enter_context(tc.tile_pool(name="fq_pool", bufs=1))
        fq = fq_pool.tile([P, FT, TSH], dt.float8e4)

        w13_pool = ctx.enter_context(tc.tile_pool(name="w13", bufs=3))
        w2_pool = ctx.enter_context(tc.tile_pool(name="w2p", bufs=2))
        ps_pool = ctx.enter_context(tc.tile_pool(name="ps", bufs=2, space="PSUM"))
        ps3_pool = ctx.enter_context(tc.tile_pool(name="ps3", bufs=2, space="PSUM"))
        ep_pool = ctx.enter_context(tc.tile_pool(name="epil", bufs=3))
        o_pool = ctx.enter_context(tc.tile_pool(name="outp", bufs=3))

        # ---- phase 1: fused^T tiles (silu(w1o)*w3o quantized) ----
        for ft in range(FT):
            w1t = w13_pool.tile([P, HCH, 2, P], dt.float8e4, tag="w1t")
            nc.sync.dma_start(w1t[:], w1_d[ft])
            w3t = w13_pool.tile([P, HCH, 2, P], dt.float8e4, tag="w3t")
            nc.sync.dma_start(w3t[:], w3_d[ft])

            ps1 = ps_pool.tile([P, TSH], dt.float32, tag="ps1")
            ps2 = ps_pool.tile([P, TSH], dt.float32, tag="ps2")
            for c in range(HCH):
                nc.tensor.matmul(ps1[:], w1t[:, c], xq_s[:, c],
                                 start=(c == 0), stop=(c == HCH - 1), perf_mode=DR)
            for c in range(HCH):
                nc.tensor.matmul(ps2[:], w3t[:, c], xq_s[:, c],
                                 start=(c == 0), stop=(c == HCH - 1), perf_mode=DR)

            sig = ep_pool.tile([P, TSH], dt.float32, tag="sig")
            nc.scalar.activation(sig[:], ps1[:], AFT.Sigmoid, scale=c_sig)
            t1 = ep_pool.tile([P, TSH], dt.float32, tag="t1")
            # only one non-scalar PSUM input allowed per DVE instruction
            nc.vector.tensor_tensor(t1[:], sig[:], ps1[:], AluOpType.mult)
            h = ep_pool.tile([P, TSH], dt.float32, tag="h")
            nc.vector.scalar_tensor_tensor(h[:], t1[:], c_prod, ps2[:],
                                           AluOpType.mult, AluOpType.mult)
            habs = ep_pool.tile([P, TSH], dt.float32, tag="habs")
            nc.scalar.activation(habs[:], h[:], AFT.Abs)
            mask = ep_pool.tile([P, TSH], dt.uint32, tag="mask")
            nc.vector.tensor_scalar(mask[:], habs[:], thresh, None, AluOpType.is_gt)
            nc.vector.tensor_copy(fq[:, ft], h[:])
            nc.vector.copy_predicated(fq[:, ft], mask[:], nan_t[:])

        if debug_fq:
            fq_d = dram.tile([P, FT, TSH], dt.float8e4, kind="ExternalOutput",
                             name="fq_dbg", uniquify=False)
            nc.sync.dma_start(fq_d[:], fq[:])

        # ---- phase 2: out^T = (fq @ (w2/2)) * c_out ----
        for ht in range(HT):
            w2t = w2_pool.tile([P, FCH, 2, P], dt.float8e4, tag="w2t")
            nc.sync.dma_start(w2t[:], w2_d[ht])
            ps3 = ps3_pool.tile([P, TSH], dt.float32, tag="ps3")
            for c in range(FCH):
                nc.tensor.matmul(ps3[:], w2t[:, c], fq[:, 2 * c:2 * c + 2],
                                 start=(c == 0), stop=(c == FCH - 1), perf_mode=DR)
            ot = o_pool.tile([P, TSH], dt.float32, tag="ot")
            nc.scalar.activation(ot[:], ps3[:], AFT.Copy, scale=c_out)
            nc.sync.dma_start(out_d[ht * P:(ht + 1) * P, :], ot[:])

    nc.compile()
    return nc


# ---------------- host-side data preparation ----------------

def prep_x(x_q8, TSH, H, core):
    """x_q8 [T, H] e4m3fn (already /xdiv'd) -> [P, H//256, 2, TSH] chunk layout."""
    xc = x_q8[core * TSH:(core + 1) * TSH]            # [TSH, H]
    a = xc.reshape(TSH, H // 256, 2, P)               # [t, c, j, r]
    return np.ascontiguousarray(a.transpose(3, 1, 2, 0))


def prep_w13(w, F, H):
    """w [F, H] f32 -> (w/2) as TRN fp8 in [F//P, P, H//256, 2, P]."""
    wh = (w.astype(np.float32) * 0.5).astype(E4)
    a = wh.reshape(F // P, P, H // 256, 2, P)         # [ft, m, c, j, r]
    return np.ascontiguousarray(a.transpose(0, 4, 2, 3, 1))


def prep_w2(w2, HID, F):
    """w2 [HID, F] f32 -> (w2/2) as TRN fp8 in [HID//P, P, F//256, 2, P]."""
    wh = (w2.astype(np.float32) * 0.5).astype(E4)
    a = wh.reshape(HID // P, P, F // 256, 2, P)       # [ht, m, c, j, r]
    return np.ascontiguousarray(a.transpose(0, 4, 2, 3, 1))


def make_core_inputs(x, w1, w3, w2, inp_scale, TSH, H, F, HID, n_cores):
    """Quantize + lay out all per-core DRAM inputs. Returns (in_maps, xdiv)."""
    inp = float(inp_scale)
    x_q8 = (x.astype(np.float32) / inp).astype(E4FN)
    # TRN fp8 bit patterns only agree with e4m3fn up to |v|=240; halve the
    # activations too if the quantized input exceeds that (not the case for
    # the graded inputs, where |x_q| <= ~6).
    xdiv = 1.0
    xf = x_q8.astype(np.float32)
    if np.nanmax(np.abs(xf)) > 240.0:
        xdiv = 2.0
        xf = xf * 0.5
    x_trn = xf.astype(E4)

    w1p = prep_w13(w1, F, H)
    w3p = prep_w13(w3, F, H)
    w2p = prep_w2(w2, HID, F)
    in_maps = []
    for c in range(n_cores):
        in_maps.append({
            "xq": prep_x(x_trn, TSH, H, c),
            "w1": w1p,
            "w3": w3p,
            "w2": w2p,
        })
    return in_maps, xdiv


def compute_constants(inp_scale, w1_scale, w2_scale, w3_scale, xdiv):
    inp = float(inp_scale)
    s1, s2, s3 = float(w1_scale), float(w2_scale), float(w3_scale)
    c_sig = 2.0 * xdiv * inp * s1          # psum1 -> w1o
    c3 = 2.0 * xdiv * inp * s3             # psum2 -> w3o
    c_prod = c_sig * c3 / (2.0 * inp)      # psum1*psum2*sig -> fused/(2*inp)
    thresh = 232.0                         # |fused/(2*inp)| above this -> NaN
    c_out = 4.0 * inp * s2                 # psum3 -> out
    return c_sig, c_prod, thresh, c_out


_PROGRAM_CACHE = {}


def _get_program(key, *args):
    if key not in _PROGRAM_CACHE:
        _PROGRAM_CACHE[key] = build_program(*args)
    return _PROGRAM_CACHE[key]


def kernel(hidden_states, w1_fp8, w3_fp8, w2_fp8,
           inp_scale, w1_scale, w2_scale, w3_scale):
    from concourse.bass_utils import run_bass_kernel_spmd

    T, H = hidden_states.shape
    F = w1_fp8.shape[0]
    HID = w2_fp8.shape[0]
    n_cores = N_CORES
    TSH = T // n_cores

    in_maps, xdiv = make_core_inputs(
        hidden_states, w1_fp8, w3_fp8, w2_fp8, inp_scale, TSH, H, F, HID, n_cores)
    c_sig, c_prod, thresh, c_out = compute_constants(
        inp_scale, w1_scale, w2_scale, w3_scale, xdiv)

    key = (TSH, H, F, HID, c_sig, c_prod, thresh, c_out)
    nc = _get_program(key, TSH, H, F, HID, c_sig, c_prod, thresh, c_out)

    res = run_bass_kernel_spmd(nc, in_maps, list(range(n_cores)))
    out = np.empty((T, HID), np.float32)
    for c in range(n_cores):
        out[c * TSH:(c + 1) * TSH, :] = res.results[c]["out"].T
    return out
